# revision 1
# baseline (speedup 1.0000x reference)
"""KanMxN fused B-spline kernel for 8 Trainium2 NeuronCores.

Math: out[b,o] = sum_{i,p} basis[i,b,p] * coeff[i,p,o] with basis the
degree-3 B-spline basis on a UNIFORM extended knot vector on [0,1]
(n_params=16, intervals=13). On a uniform knot vector every basis
function is a shifted cardinal cubic B-spline:
    basis[i,b,p] = B(13*x[i,b] + 3 - p),
    B(s) = (relu(2-|s-2|)^3 - 4*relu(1-|s-2|)^3) / 6.
So with A[(i,p), b] = 6*basis and C[(i,p), o] = coeff/6 the whole module
is one dense matmul with contraction K = n_in*n_params = 4096.

Per-core (batch-sharded 8 ways, B_local=512):
  1. DMA x slice, broadcast-replicate each x row to 16 partitions (p).
  2. ACT pass1: a = Abs(13*x_rep + (1-p))   [per-partition bias]
     ACT pass2: r2 = Relu(2 - a)
     DVE custom op (8 ALU ops): A = r2^3 - 4*relu(r2-1)^3
  3. 64 fp32r matmuls (M=o 2 tiles, N=b 512, K accumulated over 32
     tiles of 128) into 2 PSUM banks -> out^T [o=256, b=512].
  4. Copy PSUM->SBUF, DMA out. Host transposes/concats the 8 shards.
"""

import numpy as np

N_IN, N_OUT, N_PARAMS, BATCH = 256, 256, 16, 4096
NCORES = 8
BL = BATCH // NCORES          # 512 batch per core
K = N_IN * N_PARAMS           # 4096 contraction
NKT = K // 128                # 32 K-tiles
GROUPS = 8
KT_PER_G = NKT // GROUPS      # 4

# ---------------------------------------------------------------- custom DVE ops
_S1 = float(np.float32(4.0) ** (np.float32(1.0) / 3.0))
_OPS_READY = {}


def _register_ops():
    if _OPS_READY:
        return _OPS_READY
    from concourse.dve_ops import OPS, DveOp, _SUB_OPCODE_FOR_NAME, _CUSTOM_DVE_ROW_BASE
    from concourse.dve_spec import Spec, Src0, C0, C1, relu, sq, minn, lower
    from concourse.dve_uop import DveOpSpec

    def make(name, spec):
        for existing in OPS:
            if existing.name == name:
                return existing
        shas = {}
        for ver in ("v3", "v4"):
            shas[ver] = DveOpSpec(
                name=name, opcode=0, uops=lower(spec, ver=ver), rd1_en=False
            ).sha(ver)
        op = DveOp(name, spec, subdim=False, uops_sha=shas)
        OPS.append(op)
        _SUB_OPCODE_FOR_NAME[name] = _CUSTOM_DVE_ROW_BASE + len(OPS) - 1
        assert _SUB_OPCODE_FOR_NAME[name] < 0x20
        return op

    def tail_ref(in0, in1, s0, s1, imm2):
        r2 = in0.astype(np.float32)
        c2 = r2 * r2 * r2
        r1s = np.maximum(r2 * np.float32(s1) - np.float32(s1), np.float32(0.0))
        return (c2 - r1s * r1s * r1s).astype(np.float32)

    def head_ref(in0, in1, s0, s1, imm2):
        u = in0.astype(np.float32)
        return np.maximum(np.minimum(s0 - u, u + s1), np.float32(0.0)).astype(np.float32)

    _m = Src0 * C1
    _r1s = relu(_m - C1)
    tail_body = sq(Src0) * Src0 - sq(_r1s) * _r1s
    _OPS_READY["tail"] = make("KAN_TAIL_ANT", Spec(body=tail_body, reference=tail_ref))
    head_body = relu(minn(C0 - Src0, Src0 + C1))
    _OPS_READY["head"] = make("KAN_HEAD_ANT", Spec(body=head_body, reference=head_ref))
    return _OPS_READY


# ------------------------------------------------------- walrus wait-limit post-pass
def _split_sync_waits(nc, max_waits=1):
    """CoreV3 CTRL instructions (Drain) accept few sem waits; hoist extras
    onto preceding NoOps on the same engine."""
    from concourse import mybir

    for f in nc.m.functions:
        for b in f.blocks:
            new_insts = []
            for inst in b.instructions:
                si = inst.sync_info
                if si is not None and si.on_wait and len(si.on_wait) > max_waits:
                    waits = list(si.on_wait)
                    extra, keep = waits[:-max_waits], waits[-max_waits:]
                    for ci in range(0, len(extra), max_waits):
                        chunk = extra[ci : ci + max_waits]
                        new_insts.append(
                            mybir.InstNoOp(
                                name=f"{inst.name}-ws{ci}",
                                engine=inst.engine,
                                ins=[],
                                outs=[],
                                sync_info=mybir.SyncInfo(on_wait=chunk, on_update=[]),
                            )
                        )
                    inst.sync_info = mybir.SyncInfo(
                        on_wait=keep, on_update=list(si.on_update or [])
                    )
                new_insts.append(inst)
            b.instructions = new_insts


# ---------------------------------------------------------------- program builder
_PROGRAM = {}


def _build_program():
    if "nc" in _PROGRAM:
        return _PROGRAM["nc"]
    import concourse.bass as bass
    import concourse.mybir as mybir
    from concourse import tile

    ops = _register_ops()
    f32 = mybir.dt.float32
    f32r = mybir.dt.float32r

    nc = bass.Bass("TRN2", target_bir_lowering=True, debug=False, num_devices=NCORES)
    x_d = nc.dram_tensor("x", [128, 2 * BL], f32, kind="ExternalInput").ap()
    coeff_d = nc.dram_tensor("coeff_sb", [128, NKT * N_OUT], f32r, kind="ExternalInput").ap()
    bias_d = nc.dram_tensor("bias_a", [128, 1], f32, kind="ExternalInput").ap()
    bias2_d = nc.dram_tensor("bias_two", [128, 1], f32, kind="ExternalInput").ap()
    bias1_d = nc.dram_tensor("bias_one", [128, 1], f32, kind="ExternalInput").ap()
    out_d = nc.dram_tensor("outT", [N_OUT, BL], f32, kind="ExternalOutput").ap()

    with tile.TileContext(nc) as tc:
        with (
            tc.tile_pool(name="static", bufs=1) as static_pool,
            tc.tile_pool(name="dense", bufs=2) as dense_pool,
            tc.tile_pool(name="outp", bufs=1) as out_pool,
            tc.tile_pool(name="psum", bufs=1, space="PSUM") as psum_pool,
        ):
            x_sb = static_pool.tile([128, 2 * BL], f32, tag="x")
            nc.sync.dma_start(out=x_sb[:], in_=x_d[:])
            coeff_sb = static_pool.tile([128, NKT * N_OUT], f32r, tag="coeff")
            nc.sync.dma_start(out=coeff_sb[:], in_=coeff_d[:])
            bias_sb = static_pool.tile([128, 1], f32, tag="bias")
            nc.sync.dma_start(out=bias_sb[:], in_=bias_d[:])
            bias2_sb = static_pool.tile([128, 1], f32, tag="bias2")
            nc.sync.dma_start(out=bias2_sb[:], in_=bias2_d[:])
            bias1_sb = static_pool.tile([128, 1], f32, tag="bias1")
            nc.sync.dma_start(out=bias1_sb[:], in_=bias1_d[:])

            ps = [psum_pool.tile([128, BL], f32, tag=f"ps{ot}", name=f"ps{ot}") for ot in range(2)]

            for g in range(GROUPS):
                W = KT_PER_G * BL
                urep = dense_pool.tile([128, W], f32, tag="urep")
                # replicate x rows: partition (il,p) of K-tile kt <- x row kt*8+il
                for ktl in range(KT_PER_G):
                    kt = g * KT_PER_G + ktl
                    r0 = (kt * 8) % 128
                    c0 = (kt // 16) * BL
                    src = x_sb[r0 : r0 + 8, c0 : c0 + BL]
                    dst = urep[:, ktl * BL : (ktl + 1) * BL].rearrange(
                        "(il p) b -> il p b", p=N_PARAMS
                    )
                    for p in range(N_PARAMS):
                        nc.sync.dma_start(out=dst[:, p, :], in_=src)

                # a = |13x + (1-p)|
                a_t = dense_pool.tile([128, W], f32, tag="a")
                nc.scalar.activation(
                    a_t[:], urep[:], mybir.ActivationFunctionType.Abs,
                    bias=bias_sb[:], scale=13.0,
                )
                # r2 = relu(2 - a), r1 = relu(1 - a)   (ACT, per-tile bias tiles)
                r2_t = dense_pool.tile([128, W], f32, tag="r2")
                nc.scalar.activation(
                    r2_t[:], a_t[:], mybir.ActivationFunctionType.Relu,
                    bias=bias2_sb[:], scale=-1.0,
                )
                r1_t = dense_pool.tile([128, W], f32, tag="r1")
                nc.scalar.activation(
                    r1_t[:], a_t[:], mybir.ActivationFunctionType.Relu,
                    bias=bias1_sb[:], scale=-1.0,
                )
                # q2 = r2^2, q1 = r1^2 (ACT Square)
                q2_t = dense_pool.tile([128, W], f32, tag="q2")
                nc.scalar.activation(q2_t[:], r2_t[:], mybir.ActivationFunctionType.Square)
                q1_t = dense_pool.tile([128, W], f32, tag="q1")
                nc.scalar.activation(q1_t[:], r1_t[:], mybir.ActivationFunctionType.Square)
                # c2 = q2*r2 ; cm = -4*q1*r1 ; A = c2 + cm   (DVE fused ops)
                c2_t = dense_pool.tile([128, W], f32, tag="c2")
                nc.vector.scalar_tensor_tensor(
                    c2_t[:], q2_t[:], 1.0, r2_t[:],
                    op0=mybir.AluOpType.mult, op1=mybir.AluOpType.mult,
                )
                cm_t = dense_pool.tile([128, W], f32, tag="cm")
                nc.vector.scalar_tensor_tensor(
                    cm_t[:], q1_t[:], -4.0, r1_t[:],
                    op0=mybir.AluOpType.mult, op1=mybir.AluOpType.mult,
                )
                A_t = dense_pool.tile([128, W], f32r, tag="A")
                nc.vector.tensor_add(A_t[:], c2_t[:], cm_t[:])

                for ktl in range(KT_PER_G):
                    kt = g * KT_PER_G + ktl
                    rhs = A_t[:, ktl * BL : (ktl + 1) * BL]
                    for ot in range(2):
                        lhsT = coeff_sb[
                            :, kt * N_OUT + ot * 128 : kt * N_OUT + ot * 128 + 128
                        ]
                        nc.tensor.matmul(
                            ps[ot][:], lhsT, rhs, start=(kt == 0), stop=(kt == NKT - 1)
                        )

            for ot in range(2):
                o_sb = out_pool.tile([128, BL], f32, tag=f"o{ot}", name=f"o{ot}")
                nc.scalar.copy(o_sb[:], ps[ot][:])
                nc.sync.dma_start(out=out_d[ot * 128 : (ot + 1) * 128, :], in_=o_sb[:])

    _split_sync_waits(nc, max_waits=1)
    _PROGRAM["nc"] = nc
    return nc


# ---------------------------------------------------------------- host wrapper
def kernel(x, coeff, _trace=False):
    x = np.ascontiguousarray(x, dtype=np.float32)
    coeff = np.ascontiguousarray(coeff, dtype=np.float32)
    assert x.shape == (N_IN, BATCH) and coeff.shape == (N_IN, N_PARAMS, N_OUT)

    from concourse.bass_utils import run_bass_kernel_spmd

    nc = _build_program()

    # coeff/6 arranged as lhsT tiles: sbuf row r, col kt*256+o = coeff6[(kt*128+r)//16, (kt*128+r)%16, o]
    coeff6 = (coeff.astype(np.float64) / 6.0).astype(np.float32)
    coeffT = coeff6.reshape(K, N_OUT)
    coeff_sb = np.ascontiguousarray(
        coeffT.reshape(NKT, 128, N_OUT).transpose(1, 0, 2).reshape(128, NKT * N_OUT)
    )
    # per-partition ACT bias: row r = (il, p) -> 1 - p
    bias_a = np.ascontiguousarray(
        (1.0 - (np.arange(128) % N_PARAMS)).reshape(128, 1).astype(np.float32)
    )
    in_maps = []
    for c in range(NCORES):
        xs = x[:, c * BL : (c + 1) * BL]  # [256, BL]
        x_sb = np.ascontiguousarray(
            np.concatenate([xs[:128, :], xs[128:, :]], axis=1)
        )  # [128, 2*BL]: row r col t*BL+b = x[t*128+r, b]
        in_maps.append({"x": x_sb, "coeff_sb": coeff_sb, "bias_a": bias_a,
                        "bias_two": np.full((128, 1), 2.0, dtype=np.float32),
                        "bias_one": np.full((128, 1), 1.0, dtype=np.float32)})

    res = run_bass_kernel_spmd(nc, in_maps, list(range(NCORES)), trace=_trace)
    out = np.empty((BATCH, N_OUT), dtype=np.float32)
    for c in range(NCORES):
        out[c * BL : (c + 1) * BL, :] = res.results[c]["outT"].T
    if _trace:
        return out, res
    return out

